# revision 16
# baseline (speedup 1.0000x reference)
"""BNN MNIST MLP on 8 Trainium2 NeuronCores — pure data parallel, v2d.

Model (inference): x[B,784] -> relu(x @ sign(W1)) -> BN1 -> sign ->
@ sign(W2) relu BN2 sign -> @ sign(W3) -> softmax.

Design (vs the 118us v1 baseline):
  * 25% less HBM traffic: x ships as fp16 hi (2B) + fp8-e5m2 residual
    (1B, scaled 2^12; the matching stationary sign(W1)*2^-12 is exact in
    e5m2).  Validated rel_err 8.45e-3 vs the 2e-2 gate.
  * Input fully SBUF-resident; every load issued up-front, unthrottled.
  * DMA efficiency: per-engine throughput ~ bytes/(45ns + bytes/24GBps),
    so lines must be >=4KB: hi ships as slab-PAIR chunk tiles
    ([128, 2048] fp16 -> 4KB lines) and lo as full-row chunk tiles
    ([128, 8192] fp8 -> 8KB lines).  128-row chunks (6x128+16) keep all
    partitions loaded.
  * lo tiles are spread across all three DMA rings and emitted early so
    the last-consumed bytes are hi of the last slab pair, not lo: kills
    the 18us drain tail of the single-queue version.  ps1 gets 4 PSUM
    banks so early slabs hold accumulations open while lo streams in.
  * Strict ring/engine separation: sync + scalar + gpsimd rings carry
    loads only (plus gpsimd the tiny stores), DVE does all elementwise
    work (is_ge binarize in {1,0} encoding, exact via threshold folding
    t2' = (t2+colsum(W2b))/2 and host-side logits = 2*lg' - colsum(W3b)),
    PE only matmuls.  ScalarE runs nothing per-slab.
  * Layer 3 emits class-major logits ([10, B], half-integers, exact in
    fp16); softmax + transpose on host.  No PE transpose pass.
  * Column tiling: each slab's two 512-col groups run concurrently on
    PE array columns 0-63 / 64-127.
"""
import numpy as np
import ml_dtypes

import concourse.mybir as mybir
from concourse import bacc
from concourse.tile import TileContext
from concourse.bass_utils import run_bass_kernel_spmd

F32 = mybir.dt.float32
F16 = mybir.dt.float16
F8 = mybir.dt.float8e5
GE = mybir.AluOpType.is_ge

B = 65536
NCORES = 8
PER = B // NCORES          # 8192 rows per core
SLAB = 1024                # rows per slab (2 column-tiled groups of 512)
NSLAB = PER // SLAB        # 8
NPAIR = NSLAB // 2         # hi tiles cover slab pairs (4KB lines)
PAIRW = 2 * SLAB
GRP = 512
K = 784
KCS = [128, 128, 128, 128, 128, 128, 16]   # contraction chunks (6x128+16)
KOF = [0, 128, 256, 384, 512, 640, 768]
NKC = len(KCS)
NCLS = 10
NHID = 50
LOSC = 4096.0              # residual scale 2^12

EPS = 1e-3

_CACHE = {}


def _build():
    nc = bacc.Bacc("TRN2", target_bir_lowering=False, debug=False,
                   num_devices=NCORES)

    xhi = nc.dram_tensor("xhi", [K, PER], F16, kind="ExternalInput").ap()
    xlo = nc.dram_tensor("xlo", [K, PER], F8, kind="ExternalInput").ap()
    cb16 = nc.dram_tensor("cb16", [128, NKC * NHID + NHID + NCLS], F16,
                          kind="ExternalInput").ap()
    cb8 = nc.dram_tensor("cb8", [128, NKC * NHID], F8,
                         kind="ExternalInput").ap()
    # col 0 = T1, col 1 = T2eff (both replicated at partition offset 64)
    cb32 = nc.dram_tensor("cb32", [128, 2], F32, kind="ExternalInput").ap()
    out16 = nc.dram_tensor("out16", [NCLS, PER], F16,
                           kind="ExternalOutput").ap()

    with TileContext(nc) as tc:
        with (
            tc.tile_pool(name="consts", bufs=1) as cpool,
            tc.tile_pool(name="xin", bufs=1) as xpool,
            tc.tile_pool(name="mid", bufs=3) as mpool,
            tc.tile_pool(name="fin", bufs=2) as fpool,
            tc.tile_pool(name="psA", bufs=4, space="PSUM") as psA,
            tc.tile_pool(name="psB", bufs=2, space="PSUM") as psB,
            tc.tile_pool(name="psC", bufs=2, space="PSUM") as psC,
        ):
            cb16t = cpool.tile([128, NKC * NHID + NHID + NCLS], F16, tag="cb16")
            nc.scalar.dma_start(cb16t[:], cb16[:, :])
            cb8t = cpool.tile([128, NKC * NHID], F8, tag="cb8")
            nc.scalar.dma_start(cb8t[:], cb8[:, :])
            cb32t = cpool.tile([128, 2], F32, tag="cb32")
            nc.scalar.dma_start(cb32t[:], cb32[:, :])

            w1h = [cb16t[0:KCS[c], c * NHID:(c + 1) * NHID]
                   for c in range(NKC)]
            w1l = [cb8t[0:KCS[c], c * NHID:(c + 1) * NHID]
                   for c in range(NKC)]
            w2t = cb16t[0:NHID, NKC * NHID:NKC * NHID + NHID]
            w2t64 = cb16t[64:64 + NHID, NKC * NHID:NKC * NHID + NHID]
            w3t = cb16t[0:NHID, NKC * NHID + NHID:NKC * NHID + NHID + NCLS]
            w3t64 = cb16t[64:64 + NHID,
                          NKC * NHID + NHID:NKC * NHID + NHID + NCLS]
            t1t = cb32t[0:64 + NHID, 0:1]
            t2t = cb32t[0:64 + NHID, 1:2]

            # loads only ever on the two HWDGE rings (the gpsimd SWDGE
            # queue tops out at ~62GB/s and straggles 20us past them).
            # Strict alternation keeps both rings' delivery fronts on the
            # PE's consumption order.
            rings = [nc.sync, nc.scalar]
            rr = [0]

            def pick(sz):
                r = rr[0] & 1
                rr[0] += 1
                return rings[r]

            # hi tile groups: slab pairs (4KB lines) except the last two
            # slabs individually, so the PE tail starts as soon as possible.
            HGRP = [(0, 2), (2, 2), (4, 2), (6, 1), (7, 1)]
            hg = {}

            def load_hi(g):
                s0, ns = HGRP[g]
                b0, w = s0 * SLAB, ns * SLAB
                for c in range(NKC):
                    t_ = xpool.tile([KCS[c], w], F16, tag=f"xh{g}_{c}")
                    eng = pick(KCS[c] * w * 2)
                    eng.dma_start(t_[:], xhi[KOF[c]:KOF[c] + KCS[c],
                                             b0:b0 + w])
                    hg[g, c] = t_

            load_hi(0)
            # all lo tiles early, spread across the rings
            lot = []
            for c in range(NKC):
                t_ = xpool.tile([KCS[c], PER], F8, tag=f"xl{c}")
                eng = pick(KCS[c] * PER)
                eng.dma_start(t_[:], xlo[KOF[c]:KOF[c] + KCS[c], :])
                lot.append(t_)
            for g in range(1, len(HGRP)):
                load_hi(g)

            def hi_slice(s, c):
                for g, (s0, ns) in enumerate(HGRP):
                    if s0 <= s < s0 + ns:
                        off = (s - s0) * SLAB
                        return hg[g, c][:, off:off + SLAB]
                raise AssertionError

            s1t = {}
            s2t = {}
            # all logits accumulate into one resident tile; 2 stores at end
            lgall = fpool.tile([64 + NCLS, NSLAB * GRP], F16, tag="lgall")

            def stageA(s):
                ps1 = psA.tile([128, GRP], F32, tag="ps1")
                for c in range(NKC):
                    st = (c == 0)
                    xin = hi_slice(s, c)
                    nc.tensor.matmul(ps1[0:NHID, :], w1h[c], xin[:, 0:GRP],
                                     start=st, stop=False,
                                     skip_group_check=True)
                    nc.tensor.matmul(ps1[64:64 + NHID, :], w1h[c],
                                     xin[:, GRP:2 * GRP],
                                     start=st, stop=False,
                                     skip_group_check=True)
                b0 = s * SLAB
                for c in range(NKC):
                    sp = (c == NKC - 1)
                    nc.tensor.matmul(ps1[0:NHID, :], w1l[c],
                                     lot[c][:, b0:b0 + GRP],
                                     start=False, stop=sp,
                                     skip_group_check=True)
                    nc.tensor.matmul(ps1[64:64 + NHID, :], w1l[c],
                                     lot[c][:, b0 + GRP:b0 + SLAB],
                                     start=False, stop=sp,
                                     skip_group_check=True)
                s1 = mpool.tile([64 + NHID, GRP], F16, tag="s1",
                                name=f"s1_{s}")
                nc.vector.tensor_scalar(s1[:], ps1[0:64 + NHID, :],
                                        t1t, None, GE)
                s1t[s] = s1

            def stageB(p):
                ps2 = psB.tile([128, GRP], F32, tag="ps2")
                s1 = s1t[p]
                nc.tensor.matmul(ps2[0:NHID, :], w2t, s1[0:NHID, :],
                                 start=True, stop=True, skip_group_check=True)
                nc.tensor.matmul(ps2[64:64 + NHID, :], w2t64,
                                 s1[64:64 + NHID, :],
                                 start=True, stop=True, skip_group_check=True)
                s2 = mpool.tile([64 + NHID, GRP], F16, tag="s2",
                                name=f"s2_{p}")
                nc.vector.tensor_scalar(s2[:], ps2[0:64 + NHID, :],
                                        t2t, None, GE)
                s2t[p] = s2

            def stageC(p):
                ps3 = psC.tile([74, GRP], F32, tag="ps3")
                s2 = s2t[p]
                nc.tensor.matmul(ps3[0:NCLS, :], w3t, s2[0:NHID, :],
                                 start=True, stop=True, skip_group_check=True)
                nc.tensor.matmul(ps3[64:64 + NCLS, :], w3t64,
                                 s2[64:64 + NHID, :],
                                 start=True, stop=True, skip_group_check=True)
                b0 = p * GRP
                nc.vector.tensor_scalar_add(lgall[0:NCLS, b0:b0 + GRP],
                                            ps3[0:NCLS, :], 0.0)
                nc.vector.tensor_scalar_add(lgall[64:64 + NCLS, b0:b0 + GRP],
                                            ps3[64:64 + NCLS, :], 0.0)

            for p in range(NSLAB):
                stageA(p)
                if p >= 1:
                    stageB(p - 1)
                if p >= 2:
                    stageC(p - 2)
            stageB(NSLAB - 1)
            stageC(NSLAB - 2)
            stageC(NSLAB - 1)

            # group0 logits (partitions 0-9) are samples [s*1024, s*1024+512),
            # group1 (partitions 64-73) the odd 512-blocks
            ov = out16[:, :].rearrange("q (s r) -> q s r", s=NSLAB)
            nc.sync.dma_start(
                ov[:, :, 0:GRP],
                lgall[0:NCLS, :].rearrange("q (s j) -> q s j", s=NSLAB))
            nc.scalar.dma_start(
                ov[:, :, GRP:SLAB],
                lgall[64:64 + NCLS, :].rearrange("q (s j) -> q s j", s=NSLAB))

    nc.compile()
    return nc


def _prep_host(inputs, W1, W2, W3, g1, b1, m1, v1, g2, b2, m2, v2):
    x = np.ascontiguousarray(inputs.reshape(B, K).astype(np.float32,
                                                         copy=False))
    xhi = x.astype(np.float16)
    xlo = ((x - xhi.astype(np.float32)) * LOSC).astype(ml_dtypes.float8_e5m2)

    w1b = np.where(W1 >= 0, 1.0, -1.0).astype(np.float16)
    w2b = np.where(W2 >= 0, 1.0, -1.0).astype(np.float16)
    w3b = np.where(W3 >= 0, 1.0, -1.0).astype(np.float16)
    w1l = (w1b.astype(np.float32) / LOSC).astype(ml_dtypes.float8_e5m2)

    def thresh(g, b, m, v):
        a = g.astype(np.float64) / np.sqrt(v.astype(np.float64) + EPS)
        c = b.astype(np.float64) - a * m.astype(np.float64)
        t = -c / a
        return np.where(t > 0, t, -1e30).astype(np.float32)

    T1 = thresh(g1, b1, m1, v1)
    T2 = thresh(g2, b2, m2, v2)
    c2 = w2b.astype(np.float32).sum(axis=0)
    T2e = ((T2 + c2) / 2).astype(np.float32)

    cb16 = np.zeros((128, NKC * NHID + NHID + NCLS), dtype=np.float16)
    cb8 = np.zeros((128, NKC * NHID), dtype=ml_dtypes.float8_e5m2)
    for c in range(NKC):
        cb16[:KCS[c], c * NHID:(c + 1) * NHID] = w1b[KOF[c]:KOF[c] + KCS[c]]
        cb8[:KCS[c], c * NHID:(c + 1) * NHID] = w1l[KOF[c]:KOF[c] + KCS[c]]
    cb16[:NHID, NKC * NHID:NKC * NHID + NHID] = w2b
    cb16[:NHID, NKC * NHID + NHID:] = w3b
    cb16[64:64 + NHID, NKC * NHID:NKC * NHID + NHID] = w2b
    cb16[64:64 + NHID, NKC * NHID + NHID:] = w3b
    cb32 = np.zeros((128, 2), dtype=np.float32)
    cb32[:NHID, 0] = T1
    cb32[64:64 + NHID, 0] = T1
    cb32[:NHID, 1] = T2e
    cb32[64:64 + NHID, 1] = T2e
    shared = {"cb16": cb16, "cb8": cb8, "cb32": cb32}
    in_maps = []
    for c in range(NCORES):
        sl = slice(c * PER, (c + 1) * PER)
        m = dict(shared)
        m["xhi"] = np.ascontiguousarray(xhi[sl].T)
        m["xlo"] = np.ascontiguousarray(xlo[sl].T)
        in_maps.append(m)
    return in_maps


def kernel(**inputs):
    if "nc" not in _CACHE:
        _CACHE["nc"] = _build()
    nc = _CACHE["nc"]
    inputs = {k: np.asarray(v) for k, v in inputs.items()}
    in_maps = _prep_host(**inputs)
    res = run_bass_kernel_spmd(nc, in_maps, core_ids=list(range(NCORES)))
    # device logits' use {1,0} activations: true logits = 2*lg' - colsum(W3b)
    w3b = np.where(inputs["W3"] >= 0, 1.0, -1.0).astype(np.float32)
    c3 = w3b.sum(axis=0)
    lg = np.concatenate([r["out16"] for r in res.results], axis=1)
    lg = 2.0 * lg.T.astype(np.float32) - c3                  # [B, 10]
    e = np.exp(lg - lg.max(axis=1, keepdims=True))
    return (e / e.sum(axis=1, keepdims=True)).astype(np.float32)


# revision 21
# speedup vs baseline: 1.0445x; 1.0445x over previous
"""BNN MNIST MLP on 8 Trainium2 NeuronCores — pure data parallel, v2d.

Model (inference): x[B,784] -> relu(x @ sign(W1)) -> BN1 -> sign ->
@ sign(W2) relu BN2 sign -> @ sign(W3) -> softmax.

Design (vs the 118us v1 baseline):
  * 25% less HBM traffic: x ships as fp16 hi (2B) + fp8-e5m2 residual
    (1B, scaled 2^12; the matching stationary sign(W1)*2^-12 is exact in
    e5m2).  Validated rel_err 8.45e-3 vs the 2e-2 gate.
  * Input fully SBUF-resident; every load issued up-front, unthrottled.
  * DMA efficiency: per-engine throughput ~ bytes/(45ns + bytes/24GBps),
    so lines must be >=4KB: hi ships as slab-PAIR chunk tiles
    ([128, 2048] fp16 -> 4KB lines) and lo as full-row chunk tiles
    ([128, 8192] fp8 -> 8KB lines).  128-row chunks (6x128+16) keep all
    partitions loaded.
  * lo tiles are spread across all three DMA rings and emitted early so
    the last-consumed bytes are hi of the last slab pair, not lo: kills
    the 18us drain tail of the single-queue version.  ps1 gets 4 PSUM
    banks so early slabs hold accumulations open while lo streams in.
  * Strict ring/engine separation: sync + scalar + gpsimd rings carry
    loads only (plus gpsimd the tiny stores), DVE does all elementwise
    work (is_ge binarize in {1,0} encoding, exact via threshold folding
    t2' = (t2+colsum(W2b))/2 and host-side logits = 2*lg' - colsum(W3b)),
    PE only matmuls.  ScalarE runs nothing per-slab.
  * Layer 3 emits class-major logits ([10, B], half-integers, exact in
    fp16); softmax + transpose on host.  No PE transpose pass.
  * Column tiling: each slab's two 512-col groups run concurrently on
    PE array columns 0-63 / 64-127.
"""
import numpy as np
import ml_dtypes

import concourse.mybir as mybir
from concourse import bacc
from concourse.tile import TileContext
from concourse.bass_utils import run_bass_kernel_spmd

F32 = mybir.dt.float32
F16 = mybir.dt.float16
F8 = mybir.dt.float8e5
GE = mybir.AluOpType.is_ge

B = 65536
NCORES = 8
PER = B // NCORES          # 8192 rows per core
SLAB = 1024                # rows per slab (2 column-tiled groups of 512)
NSLAB = PER // SLAB        # 8
NPAIR = NSLAB // 2         # hi tiles cover slab pairs (4KB lines)
PAIRW = 2 * SLAB
GRP = 512
K = 784
KCS = [128, 128, 128, 128, 128, 128, 16]   # contraction chunks (6x128+16)
KOF = [0, 128, 256, 384, 512, 640, 768]
NKC = len(KCS)
NCLS = 10
NHID = 50
LOSC = 4096.0              # residual scale 2^12

EPS = 1e-3

_CACHE = {}


def _build():
    nc = bacc.Bacc("TRN2", target_bir_lowering=False, debug=False,
                   num_devices=NCORES)

    xhi = nc.dram_tensor("xhi", [K, PER], F16, kind="ExternalInput").ap()
    xlo = nc.dram_tensor("xlo", [K, PER], F8, kind="ExternalInput").ap()
    cb16 = nc.dram_tensor("cb16", [128, NKC * NHID + NHID + NCLS], F16,
                          kind="ExternalInput").ap()
    cb8 = nc.dram_tensor("cb8", [128, NKC * NHID], F8,
                         kind="ExternalInput").ap()
    # col 0 = T1, col 1 = T2eff (both replicated at partition offset 64)
    cb32 = nc.dram_tensor("cb32", [128, 2], F32, kind="ExternalInput").ap()
    out16 = nc.dram_tensor("out16", [NCLS, PER], F16,
                           kind="ExternalOutput").ap()

    with TileContext(nc) as tc:
        with (
            tc.tile_pool(name="consts", bufs=1) as cpool,
            tc.tile_pool(name="xin", bufs=1) as xpool,
            tc.tile_pool(name="mid", bufs=3) as mpool,
            tc.tile_pool(name="fin", bufs=2) as fpool,
            tc.tile_pool(name="psA", bufs=4, space="PSUM") as psA,
            tc.tile_pool(name="psB", bufs=2, space="PSUM") as psB,
            tc.tile_pool(name="psC", bufs=2, space="PSUM") as psC,
        ):
            cb16t = cpool.tile([128, NKC * NHID + NHID + NCLS], F16, tag="cb16")
            nc.scalar.dma_start(cb16t[:], cb16[:, :])
            cb8t = cpool.tile([128, NKC * NHID], F8, tag="cb8")
            nc.scalar.dma_start(cb8t[:], cb8[:, :])
            cb32t = cpool.tile([128, 2], F32, tag="cb32")
            nc.scalar.dma_start(cb32t[:], cb32[:, :])

            w1h = [cb16t[0:KCS[c], c * NHID:(c + 1) * NHID]
                   for c in range(NKC)]
            w1l = [cb8t[0:KCS[c], c * NHID:(c + 1) * NHID]
                   for c in range(NKC)]
            w2t = cb16t[0:NHID, NKC * NHID:NKC * NHID + NHID]
            w2t64 = cb16t[64:64 + NHID, NKC * NHID:NKC * NHID + NHID]
            w3t = cb16t[0:NHID, NKC * NHID + NHID:NKC * NHID + NHID + NCLS]
            w3t64 = cb16t[64:64 + NHID,
                          NKC * NHID + NHID:NKC * NHID + NHID + NCLS]
            t1t = cb32t[0:64 + NHID, 0:1]
            t2t = cb32t[0:64 + NHID, 1:2]

            # loads only ever on the two HWDGE rings (the gpsimd SWDGE
            # queue tops out at ~62GB/s and straggles 20us past them).
            # Strict alternation keeps both rings' delivery fronts on the
            # PE's consumption order.
            rings = [nc.sync, nc.scalar]
            rr = [0]

            def pick(sz):
                r = rr[0] & 1
                rr[0] += 1
                return rings[r]

            # hi tile groups: slab pairs (4KB lines; singles would halve the
            # tail tiles' per-engine DMA rate and delay final delivery)
            HGRP = [(0, 2), (2, 2), (4, 2), (6, 2)]
            hg = {}

            def load_hi(g):
                s0, ns = HGRP[g]
                b0, w = s0 * SLAB, ns * SLAB
                for c in range(NKC):
                    t_ = xpool.tile([KCS[c], w], F16, tag=f"xh{g}_{c}")
                    eng = pick(KCS[c] * w * 2)
                    eng.dma_start(t_[:], xhi[KOF[c]:KOF[c] + KCS[c],
                                             b0:b0 + w])
                    hg[g, c] = t_

            load_hi(0)
            # all lo tiles early, spread across the rings (8KB lines)
            lot = []
            for c in range(NKC):
                t_ = xpool.tile([KCS[c], PER], F8, tag=f"xl{c}")
                eng = pick(KCS[c] * PER)
                eng.dma_start(t_[:], xlo[KOF[c]:KOF[c] + KCS[c], :])
                lot.append(t_)
            for g in range(1, len(HGRP)):
                load_hi(g)

            def hi_slice(s, c):
                for g, (s0, ns) in enumerate(HGRP):
                    if s0 <= s < s0 + ns:
                        off = (s - s0) * SLAB
                        return hg[g, c][:, off:off + SLAB]
                raise AssertionError

            s1t = {}
            s2t = {}
            # all logits accumulate into one resident tile; 2 stores at end
            lgall = fpool.tile([64 + NCLS, NSLAB * GRP], F16, tag="lgall")

            def stageA(s):
                ps1 = psA.tile([128, GRP], F32, tag="ps1")
                for c in range(NKC):
                    st = (c == 0)
                    xin = hi_slice(s, c)
                    nc.tensor.matmul(ps1[0:NHID, :], w1h[c], xin[:, 0:GRP],
                                     start=st, stop=False,
                                     skip_group_check=True)
                    nc.tensor.matmul(ps1[64:64 + NHID, :], w1h[c],
                                     xin[:, GRP:2 * GRP],
                                     start=st, stop=False,
                                     skip_group_check=True)
                b0 = s * SLAB
                for c in range(NKC):
                    sp = (c == NKC - 1)
                    nc.tensor.matmul(ps1[0:NHID, :], w1l[c],
                                     lot[c][:, b0:b0 + GRP],
                                     start=False, stop=sp,
                                     skip_group_check=True)
                    nc.tensor.matmul(ps1[64:64 + NHID, :], w1l[c],
                                     lot[c][:, b0 + GRP:b0 + SLAB],
                                     start=False, stop=sp,
                                     skip_group_check=True)
                s1 = mpool.tile([64 + NHID, GRP], F16, tag="s1",
                                name=f"s1_{s}")
                nc.vector.tensor_scalar(s1[:], ps1[0:64 + NHID, :],
                                        t1t, None, GE)
                s1t[s] = s1

            def stageB(p):
                ps2 = psB.tile([128, GRP], F32, tag="ps2")
                s1 = s1t[p]
                nc.tensor.matmul(ps2[0:NHID, :], w2t, s1[0:NHID, :],
                                 start=True, stop=True, skip_group_check=True)
                nc.tensor.matmul(ps2[64:64 + NHID, :], w2t64,
                                 s1[64:64 + NHID, :],
                                 start=True, stop=True, skip_group_check=True)
                s2 = mpool.tile([64 + NHID, GRP], F16, tag="s2",
                                name=f"s2_{p}")
                nc.vector.tensor_scalar(s2[:], ps2[0:64 + NHID, :],
                                        t2t, None, GE)
                s2t[p] = s2

            def stageC(p):
                ps3 = psC.tile([74, GRP], F32, tag="ps3")
                s2 = s2t[p]
                nc.tensor.matmul(ps3[0:NCLS, :], w3t, s2[0:NHID, :],
                                 start=True, stop=True, skip_group_check=True)
                nc.tensor.matmul(ps3[64:64 + NCLS, :], w3t64,
                                 s2[64:64 + NHID, :],
                                 start=True, stop=True, skip_group_check=True)
                b0 = p * GRP
                nc.vector.tensor_scalar_add(lgall[0:NCLS, b0:b0 + GRP],
                                            ps3[0:NCLS, :], 0.0)
                nc.vector.tensor_scalar_add(lgall[64:64 + NCLS, b0:b0 + GRP],
                                            ps3[64:64 + NCLS, :], 0.0)

            for p in range(NSLAB):
                stageA(p)
                if p >= 1:
                    stageB(p - 1)
                if p >= 2:
                    stageC(p - 2)
            stageB(NSLAB - 1)
            stageC(NSLAB - 2)
            stageC(NSLAB - 1)

            # group0 logits (partitions 0-9) are samples [s*1024, s*1024+512),
            # group1 (partitions 64-73) the odd 512-blocks
            ov = out16[:, :].rearrange("q (s r) -> q s r", s=NSLAB)
            nc.sync.dma_start(
                ov[:, :, 0:GRP],
                lgall[0:NCLS, :].rearrange("q (s j) -> q s j", s=NSLAB))
            nc.scalar.dma_start(
                ov[:, :, GRP:SLAB],
                lgall[64:64 + NCLS, :].rearrange("q (s j) -> q s j", s=NSLAB))

    nc.compile()
    return nc


def _prep_host(inputs, W1, W2, W3, g1, b1, m1, v1, g2, b2, m2, v2):
    x = np.ascontiguousarray(inputs.reshape(B, K).astype(np.float32,
                                                         copy=False))
    xhi = x.astype(np.float16)
    xlo = ((x - xhi.astype(np.float32)) * LOSC).astype(ml_dtypes.float8_e5m2)

    w1b = np.where(W1 >= 0, 1.0, -1.0).astype(np.float16)
    w2b = np.where(W2 >= 0, 1.0, -1.0).astype(np.float16)
    w3b = np.where(W3 >= 0, 1.0, -1.0).astype(np.float16)
    w1l = (w1b.astype(np.float32) / LOSC).astype(ml_dtypes.float8_e5m2)

    def thresh(g, b, m, v):
        a = g.astype(np.float64) / np.sqrt(v.astype(np.float64) + EPS)
        c = b.astype(np.float64) - a * m.astype(np.float64)
        t = -c / a
        return np.where(t > 0, t, -1e30).astype(np.float32)

    T1 = thresh(g1, b1, m1, v1)
    T2 = thresh(g2, b2, m2, v2)
    c2 = w2b.astype(np.float32).sum(axis=0)
    T2e = ((T2 + c2) / 2).astype(np.float32)

    cb16 = np.zeros((128, NKC * NHID + NHID + NCLS), dtype=np.float16)
    cb8 = np.zeros((128, NKC * NHID), dtype=ml_dtypes.float8_e5m2)
    for c in range(NKC):
        cb16[:KCS[c], c * NHID:(c + 1) * NHID] = w1b[KOF[c]:KOF[c] + KCS[c]]
        cb8[:KCS[c], c * NHID:(c + 1) * NHID] = w1l[KOF[c]:KOF[c] + KCS[c]]
    cb16[:NHID, NKC * NHID:NKC * NHID + NHID] = w2b
    cb16[:NHID, NKC * NHID + NHID:] = w3b
    cb16[64:64 + NHID, NKC * NHID:NKC * NHID + NHID] = w2b
    cb16[64:64 + NHID, NKC * NHID + NHID:] = w3b
    cb32 = np.zeros((128, 2), dtype=np.float32)
    cb32[:NHID, 0] = T1
    cb32[64:64 + NHID, 0] = T1
    cb32[:NHID, 1] = T2e
    cb32[64:64 + NHID, 1] = T2e
    shared = {"cb16": cb16, "cb8": cb8, "cb32": cb32}
    in_maps = []
    for c in range(NCORES):
        sl = slice(c * PER, (c + 1) * PER)
        m = dict(shared)
        m["xhi"] = np.ascontiguousarray(xhi[sl].T)
        m["xlo"] = np.ascontiguousarray(xlo[sl].T)
        in_maps.append(m)
    return in_maps


def kernel(**inputs):
    if "nc" not in _CACHE:
        _CACHE["nc"] = _build()
    nc = _CACHE["nc"]
    inputs = {k: np.asarray(v) for k, v in inputs.items()}
    in_maps = _prep_host(**inputs)
    res = run_bass_kernel_spmd(nc, in_maps, core_ids=list(range(NCORES)))
    # device logits' use {1,0} activations: true logits = 2*lg' - colsum(W3b)
    w3b = np.where(inputs["W3"] >= 0, 1.0, -1.0).astype(np.float32)
    c3 = w3b.sum(axis=0)
    lg = np.concatenate([r["out16"] for r in res.results], axis=1)
    lg = 2.0 * lg.T.astype(np.float32) - c3                  # [B, 10]
    e = np.exp(lg - lg.max(axis=1, keepdims=True))
    return (e / e.sum(axis=1, keepdims=True)).astype(np.float32)
